# revision 30
# baseline (speedup 1.0000x reference)
"""Chamfer loss kernel for Trainium2 (8 NeuronCores).

Problem: pred/target [4, 3, 8192] channel-first point clouds.
loss = mean_i min_j ||p_i - t_j|| + mean_j min_i ||p_i - t_j||

d2[i,j] = ||p_i||^2 + ||t_j||^2 - 2 p_i.t_j is expressed as a single
K=16 fp16 matmul per tile (hi/lo splits keep |err| ~1e-6).  sqrt is
monotonic, so mins are taken over d2 and sqrt'd on host.

Sharding: core c -> (batch b = c//2, pred-row half h = c%2).  Each core
computes a [4096, 8192] block of d2 as 32 row tiles x 4 chunks of
[128, 2048] in PSUM and extracts
  - row mins  (min over the 8192 cols)  -> rowmin [128, 32]
  - col mins  (partial, per-partition)  -> colacc [128, 8192] -> PE
    transpose + reduce -> colmin [128, 64]
Host combines the tiny outputs.

The post-matmul reduction is the bottleneck.  ScalarE exports each PSUM
tile to SBUF fp16 (1 elem/cycle/lane); VectorE does both min directions
on the fp16 data at its 2x packed rate: one wide tensor_tensor min per
row tile into colacc, and a 4-level pairwise fold tree + tensor_reduce
for the row mins.  fp32 accumulation and the PE-transpose finale give
the cross-partition column mins.

Notes from exploration (this toolchain, axon/walrus):
  - nc.gpsimd.tensor_tensor/"Pool TensorTensor" fails walrus codegen
    (ISA engine check) - GPSIMD cannot help with elementwise min.
  - nc.vector.tensor_tensor_reduce compiles but crashes the device
    (NRT_EXEC_UNIT_UNRECOVERABLE) in every variant tried.
  - nc.vector.pool_max runs at 1x (no packed mode) - slower than the
    2x fold tree.
  - DMA cannot read PSUM (dma_start asserts SBUF/DRAM source).
Hence all reduction work lands on DVE (~303 us/core modeled busy),
ACT ~263 us, PE ~116 us; modeled total 320 us, measured 290-325 us
(session noise is +/-15%).

Each row tile is assigned a "way"; only "E" is usable here:
  E: ACT export fp16; DVE colmin TT + rowmin fold tree
  G/F/V (GPSIMD offload) and P (PSUM-direct DVE) are kept for
  reference but fail or lose on this toolchain.
"""

import numpy as np

B = 4
D = 3
N = 8192
HALF = N // 2  # pred rows per core
NCORES = 8
K = 16  # augmented contraction dim
RT = HALF // 128  # 32 row tiles per core
GW = 2048  # cols per PSUM tile (4 banks; 2 tiles in flight)
MMW = 512  # cols per matmul (one PSUM bank)
NT = N // 128  # 64 transpose blocks in the colmin finale

# Per-row-tile strategy, len 32.  r=0 must be E (its exports init colacc).
# GPSIMD (G/F/V) and tensor_tensor_reduce are rejected by this toolchain's
# walrus/runtime, so the default is all-E with the DVE fold tree.
WAYS_DEFAULT = "E" * 32

_CACHE = {}


def _build_nc(ways=WAYS_DEFAULT, loop_n=None, rowmode="tree"):
    """loop_n: wrap the body in a device-side For_i loop executed loop_n
    times - constant program size, used for timing (delta between two
    loop_n values isolates pure HW execution time)."""
    import concourse.bacc as bacc
    import concourse.tile as tile
    from concourse import mybir

    assert len(ways) == RT and ways[0] == "E" and all(c in "EGFPV" for c in ways)
    f16 = mybir.dt.float16
    f32 = mybir.dt.float32
    MIN = mybir.AluOpType.min
    X = mybir.AxisListType.X
    BIG = 3.0e38

    uses_b = any(c in "GFV" for c in ways)

    nc = bacc.Bacc(
        "TRN2", target_bir_lowering=False, debug=False, num_devices=NCORES
    )
    stat = nc.dram_tensor("stat", [K, HALF], f16, kind="ExternalInput").ap()
    mov = nc.dram_tensor("mov", [K, N], f16, kind="ExternalInput").ap()
    ident = nc.dram_tensor("ident", [128, 128], f32, kind="ExternalInput").ap()
    rowmin_o = nc.dram_tensor("rowmin", [128, RT], f32, kind="ExternalOutput").ap()
    colmin_o = nc.dram_tensor("colmin", [128, NT], f32, kind="ExternalOutput").ap()

    with tile.TileContext(nc) as tc:
        with (
            tc.tile_pool(name="persist", bufs=1) as persist,
            tc.tile_pool(name="psum", bufs=2, space="PSUM") as psum_pool,
            tc.tile_pool(name="ckt", bufs=3) as ckt_pool,
            tc.tile_pool(name="scr", bufs=2) as scr_pool,
            tc.tile_pool(name="rp", bufs=2) as rp_pool,
        ):
            stat_sb = persist.tile([K, HALF], f16)
            mov_sb = persist.tile([K, N], f16)
            ident_sb = persist.tile([128, 128], f32)
            colacc = persist.tile([128, N], f16)
            colaccB = persist.tile([128, N], f32)
            rowmins = persist.tile([128, RT], f32)
            colmins = persist.tile([128, NT], f32)
            nc.sync.dma_start(stat_sb[:], stat)
            nc.sync.dma_start(mov_sb[:], mov)
            nc.sync.dma_start(ident_sb[:], ident)

            import contextlib

            loop_cm = (
                tc.For_i(0, loop_n, 1)
                if loop_n is not None
                else contextlib.nullcontext()
            )
            with loop_cm:
                b_inited = False
                for r, way in enumerate(ways):
                    lhsT = stat_sb[:, r * 128 : (r + 1) * 128]

                    if way == "P":
                        # PSUM-direct: both reductions read PSUM, no export.
                        rp = rp_pool.tile([128, 4], f32)
                        for g in range(4):
                            pt = psum_pool.tile([128, GW], f32, tag="pt")
                            for s in range(GW // MMW):
                                c0 = g * GW + s * MMW
                                nc.tensor.matmul(
                                    pt[:, s * MMW : (s + 1) * MMW],
                                    lhsT,
                                    mov_sb[:, c0 : c0 + MMW],
                                    start=True,
                                    stop=True,
                                )
                            csl = colacc[:, g * GW : (g + 1) * GW]
                            nc.vector.tensor_tensor(csl, pt[:], csl, MIN)
                            scr = scr_pool.tile([128, N // 2], f16)
                            nc.vector.tensor_tensor_reduce(
                                scr[:, : GW // 2],
                                pt[:, : GW // 2],
                                pt[:, GW // 2 :],
                                1.0,
                                BIG,
                                MIN,
                                MIN,
                                rp[:, g : g + 1],
                            )
                        nc.vector.tensor_reduce(
                            rowmins[:, r : r + 1], rp[:], X, MIN
                        )
                        continue

                    # Exported tiles.  First exported tile of each
                    # accumulator writes it directly (free init).
                    init_b = False
                    if way == "E" and r == 0:
                        dst = colacc
                    elif way in "GFV" and not b_inited:
                        dst = colaccB
                        b_inited = True
                        init_b = True
                    else:
                        dst = ckt_pool.tile([128, N], f16, tag="ck16")

                    for g in range(4):
                        pt = psum_pool.tile([128, GW], f32, tag="pt")
                        for s in range(GW // MMW):
                            c0 = g * GW + s * MMW
                            nc.tensor.matmul(
                                pt[:, s * MMW : (s + 1) * MMW],
                                lhsT,
                                mov_sb[:, c0 : c0 + MMW],
                                start=True,
                                stop=True,
                            )
                        dsl = dst[:, g * GW : (g + 1) * GW]
                        if way == "V":
                            nc.vector.tensor_copy(dsl, pt[:])
                        else:
                            nc.scalar.copy(dsl, pt[:])

                    # colmin merge
                    if way == "E":
                        if r > 0:
                            nc.vector.tensor_tensor(colacc[:], dst[:], colacc[:], MIN)
                    elif not init_b:  # G/F/V
                        nc.gpsimd.tensor_tensor(colaccB[:], dst[:], colaccB[:], MIN)

                    # rowmin
                    if way == "F":
                        nc.gpsimd.tensor_reduce(
                            rowmins[:, r : r + 1], dst[:], X, MIN
                        )
                    elif rowmode == "ttr":
                        scr = scr_pool.tile([128, N // 2], f16)
                        nc.vector.tensor_tensor_reduce(
                            scr[:],
                            dst[:, : N // 2],
                            dst[:, N // 2 :],
                            1.0,
                            BIG,
                            MIN,
                            MIN,
                            rowmins[:, r : r + 1],
                        )
                    elif rowmode == "ttr_bc":
                        # qr.py-style: dummy broadcast out, real accum
                        scr = scr_pool.tile([128, 1], f16, tag="scrbc")
                        nc.vector.tensor_tensor_reduce(
                            scr[:].broadcast_to((128, N // 2)),
                            dst[:, : N // 2],
                            dst[:, N // 2 :],
                            1.0,
                            BIG,
                            MIN,
                            MIN,
                            rowmins[:, r : r + 1],
                        )
                    elif rowmode == "ttr_add":
                        scr = scr_pool.tile([128, 1], f16, tag="scrbc")
                        nc.vector.tensor_tensor_reduce(
                            scr[:].broadcast_to((128, N // 2)),
                            dst[:, : N // 2],
                            dst[:, N // 2 :],
                            1.0,
                            0.0,
                            MIN,
                            mybir.AluOpType.add,
                            rowmins[:, r : r + 1],
                        )
                    elif rowmode == "ttr_rp":
                        scr = scr_pool.tile([128, N // 2], f16)
                        rp = rp_pool.tile([128, 4], f32)
                        nc.vector.tensor_tensor_reduce(
                            scr[:],
                            dst[:, : N // 2],
                            dst[:, N // 2 :],
                            1.0,
                            BIG,
                            MIN,
                            MIN,
                            rp[:, 0:1],
                        )
                        nc.vector.tensor_reduce(
                            rowmins[:, r : r + 1], rp[:, 0:1], X, MIN
                        )
                    elif rowmode == "pooltest":
                        # timing probe only: row-MAX via pool (wrong values)
                        nc.vector.pool_max(rowmins[:, r : r + 1], dst[:])
                    else:  # tree
                        scr = scr_pool.tile([128, N // 2], f16)
                        nc.vector.tensor_tensor(
                            scr[:], dst[:, : N // 2], dst[:, N // 2 :], MIN
                        )
                        scr2 = scr_pool.tile([128, N // 4], f16, tag="scr2")
                        nc.vector.tensor_tensor(
                            scr2[:], scr[:, : N // 4], scr[:, N // 4 :], MIN
                        )
                        scr3 = scr_pool.tile([128, N // 8], f16, tag="scr3")
                        nc.vector.tensor_tensor(
                            scr3[:], scr2[:, : N // 8], scr2[:, N // 8 :], MIN
                        )
                        scr4 = scr_pool.tile([128, N // 16], f16, tag="scr4")
                        nc.vector.tensor_tensor(
                            scr4[:], scr3[:, : N // 16], scr3[:, N // 16 :], MIN
                        )
                        nc.vector.tensor_reduce(
                            rowmins[:, r : r + 1], scr4[:], X, MIN
                        )

                # Fold the fp16 accumulator into the fp32 one; chunked so it
                # pipelines with the finale transposes.  The finale
                # transposes read fp32 (PE transpose out dtype must match).
                # The no-B cast-copy runs on ACT, which has slack.
                for q in range(4):
                    sl = slice(q * GW, (q + 1) * GW)
                    if uses_b:
                        nc.vector.tensor_tensor(
                            colaccB[:, sl], colacc[:, sl], colaccB[:, sl], MIN
                        )
                    else:
                        nc.scalar.copy(colaccB[:, sl], colacc[:, sl])

                # --- colmin finale: cross-partition reduce of colaccB ---
                # PE transpose of 128x128 blocks, packed min-reduce
                # 4 blocks per PSUM tile on DVE.
                for j in range(NT // 4):
                    pf = psum_pool.tile([128, GW], f32, tag="pt")
                    for kk in range(4):
                        t = 4 * j + kk
                        nc.tensor.matmul(
                            pf[:, kk * 128 : (kk + 1) * 128],
                            colaccB[:, t * 128 : (t + 1) * 128],
                            ident_sb[:],
                            is_transpose=True,
                            start=True,
                            stop=True,
                        )
                    nc.vector.tensor_reduce(
                        colmins[:, 4 * j : 4 * j + 4],
                        pf[:, :512].rearrange("p (b f) -> p b f", b=4),
                        X,
                        MIN,
                    )
            nc.sync.dma_start(rowmin_o, rowmins[:])
            nc.sync.dma_start(colmin_o, colmins[:])
    nc.compile()
    return nc


def _get_nc():
    if "nc" not in _CACHE:
        _CACHE["nc"] = _build_nc()
    return _CACHE["nc"]


def _split16(x):
    hi = x.astype(np.float16)
    lo = (x - hi.astype(np.float32)).astype(np.float16)
    return hi, lo


def _prep_batch(p, t):
    """p, t: [3, N] fp32 -> (S [K, N] fp16 stationary, M [K, N] fp16 moving)
    with d2[i, j] = sum_k S[k, i] * M[k, j] to ~1e-6 absolute."""
    p2 = (p * p).sum(axis=0)
    t2 = (t * t).sum(axis=0)
    S = np.empty((K, N), np.float16)
    M = np.empty((K, N), np.float16)
    S[0], S[1] = _split16(p2)
    M[0] = 1.0
    M[1] = 1.0
    S[2] = 1.0
    S[3] = 1.0
    M[2], M[3] = _split16(t2)
    for d in range(D):
        ah, al = _split16(-2.0 * p[d])
        bh, bl = _split16(t[d])
        base = 4 + 4 * d
        S[base + 0] = ah
        M[base + 0] = bh
        S[base + 1] = ah
        M[base + 1] = bl
        S[base + 2] = al
        M[base + 2] = bh
        S[base + 3] = al
        M[base + 3] = bl
    return S, M


def _make_in_maps(pred, target):
    pred = np.asarray(pred, dtype=np.float32)
    target = np.asarray(target, dtype=np.float32)
    ident = np.eye(128, dtype=np.float32)
    in_maps = []
    for c in range(NCORES):
        b, h = divmod(c, 2)
        S, M = _prep_batch(pred[b], target[b])
        in_maps.append(
            {
                "stat": np.ascontiguousarray(S[:, h * HALF : (h + 1) * HALF]),
                "mov": M,
                "ident": ident,
            }
        )
    return in_maps


def _finish(results):
    """results: list of 8 dicts with 'rowmin' [128, RT] f32 and
    'colmin' [128, NT] f32 (colmin[p, t] = min_i d2[i, 128*t + p])."""
    row_total = 0.0
    col_total = 0.0
    for b in range(B):
        colparts = []
        for h in range(2):
            out = results[2 * b + h]
            rm = np.asarray(out["rowmin"], dtype=np.float32)  # [128, RT]
            # row index within half = r*128 + p -> transpose to [RT, 128]
            rd2 = np.maximum(rm.T.reshape(-1), 0.0)
            row_total += np.sqrt(rd2, dtype=np.float64).sum()
            # column j = 128*t + p -> transpose [NT, 128] then flatten
            colparts.append(
                np.asarray(out["colmin"], dtype=np.float32).T.reshape(-1)
            )
        cd2 = np.maximum(np.minimum(colparts[0], colparts[1]), 0.0)
        col_total += np.sqrt(cd2, dtype=np.float64).sum()
    loss = row_total / (B * N) + col_total / (B * N)
    return np.array(loss, dtype=np.float32)


def _run(in_maps, trace=False, nc=None):
    from concourse.bass_utils import run_bass_kernel_spmd

    if nc is None:
        nc = _get_nc()
    res = run_bass_kernel_spmd(
        nc, in_maps, list(range(NCORES)), trace=trace
    )
    return res


def kernel(pred, target):
    res = _run(_make_in_maps(pred, target), trace=False)
    return _finish(res.results)


# revision 31
# speedup vs baseline: 1.0931x; 1.0931x over previous
"""Chamfer loss kernel for Trainium2 (8 NeuronCores).

Problem: pred/target [4, 3, 8192] channel-first point clouds.
loss = mean_i min_j ||p_i - t_j|| + mean_j min_i ||p_i - t_j||

d2[i,j] = ||p_i||^2 + ||t_j||^2 - 2 p_i.t_j is expressed as a single
K=16 fp16 matmul per tile (hi/lo splits keep |err| ~1e-6).  sqrt is
monotonic, so mins are taken over d2 and sqrt'd on host.

Sharding: core c -> (batch b = c//2, pred-row half h = c%2).  Each core
computes a [4096, 8192] block of d2 as 32 row tiles x 4 chunks of
[128, 2048] in PSUM and extracts
  - row mins  (min over the 8192 cols)  -> rowmin [128, 32]
  - col mins  (partial, per-partition)  -> colacc [128, 8192] -> PE
    transpose + reduce -> colmin [128, 64]
Host combines the tiny outputs.

The post-matmul reduction is the bottleneck.  ScalarE exports each PSUM
tile to SBUF fp16 (1 elem/cycle/lane); VectorE does both min directions
on the fp16 data at its 2x packed rate: one wide tensor_tensor min per
row tile into colacc, and a 4-level pairwise fold tree + tensor_reduce
for the row mins.  fp32 accumulation and the PE-transpose finale give
the cross-partition column mins.

Notes from exploration (this toolchain, axon/walrus):
  - nc.gpsimd.tensor_tensor/"Pool TensorTensor" fails walrus codegen
    (ISA engine check) - GPSIMD cannot help with elementwise min.
  - nc.vector.tensor_tensor_reduce compiles but crashes the device
    (NRT_EXEC_UNIT_UNRECOVERABLE) in every variant tried.
  - nc.vector.pool_max runs at 1x (no packed mode) - slower than the
    2x fold tree.
  - DMA cannot read PSUM (dma_start asserts SBUF/DRAM source).
Hence all reduction work lands on DVE (~303 us/core modeled busy),
ACT ~263 us, PE ~116 us; modeled total 320 us, measured 290-325 us
(session noise is +/-15%).

Each row tile is assigned a "way"; only "E" is usable here:
  E: ACT export fp16; DVE colmin TT + rowmin fold tree
  G/F/V (GPSIMD offload) and P (PSUM-direct DVE) are kept for
  reference but fail or lose on this toolchain.
"""

import numpy as np

B = 4
D = 3
N = 8192
HALF = N // 2  # pred rows per core
NCORES = 8
K = 16  # augmented contraction dim
RT = HALF // 128  # 32 row tiles per core
GW = 2048  # cols per PSUM tile (4 banks; 2 tiles in flight)
MMW = 512  # cols per matmul (one PSUM bank)
NT = N // 128  # 64 transpose blocks in the colmin finale

# Per-row-tile strategy, len 32.  r=0 must be E (its exports init colacc).
# GPSIMD (G/F/V) and tensor_tensor_reduce are rejected by this toolchain's
# walrus/runtime, so the default is all-E with the DVE fold tree.
WAYS_DEFAULT = "E" * 32

_CACHE = {}


def _build_nc(ways=WAYS_DEFAULT, loop_n=None, rowmode="tree"):
    """loop_n: wrap the body in a device-side For_i loop executed loop_n
    times - constant program size, used for timing (delta between two
    loop_n values isolates pure HW execution time)."""
    import concourse.bacc as bacc
    import concourse.tile as tile
    from concourse import mybir

    assert len(ways) == RT and ways[0] == "E" and all(c in "EGFPV" for c in ways)
    f16 = mybir.dt.float16
    f32 = mybir.dt.float32
    MIN = mybir.AluOpType.min
    X = mybir.AxisListType.X
    BIG = 3.0e38

    uses_b = any(c in "GFV" for c in ways)

    nc = bacc.Bacc(
        "TRN2", target_bir_lowering=False, debug=False, num_devices=NCORES
    )
    stat = nc.dram_tensor("stat", [K, HALF], f16, kind="ExternalInput").ap()
    mov = nc.dram_tensor("mov", [K, N], f16, kind="ExternalInput").ap()
    ident = nc.dram_tensor("ident", [128, 128], f32, kind="ExternalInput").ap()
    rowmin_o = nc.dram_tensor("rowmin", [128, RT], f32, kind="ExternalOutput").ap()
    colmin_o = nc.dram_tensor("colmin", [128, NT], f32, kind="ExternalOutput").ap()

    with tile.TileContext(nc) as tc:
        with (
            tc.tile_pool(name="persist", bufs=1) as persist,
            tc.tile_pool(name="psum", bufs=2, space="PSUM") as psum_pool,
            tc.tile_pool(name="ckt", bufs=3) as ckt_pool,
            tc.tile_pool(name="scr", bufs=2) as scr_pool,
            tc.tile_pool(name="rp", bufs=2) as rp_pool,
        ):
            stat_sb = persist.tile([K, HALF], f16)
            mov_sb = persist.tile([K, N], f16)
            ident_sb = persist.tile([128, 128], f32)
            colacc = persist.tile([128, N], f16)
            colaccB = persist.tile([128, N], f32)
            rowmins = persist.tile([128, RT], f32)
            colmins = persist.tile([128, NT], f32)
            nc.sync.dma_start(stat_sb[:], stat)
            nc.sync.dma_start(mov_sb[:], mov)
            nc.sync.dma_start(ident_sb[:], ident)

            import contextlib

            loop_cm = (
                tc.For_i(0, loop_n, 1)
                if loop_n is not None
                else contextlib.nullcontext()
            )
            with loop_cm:
                b_inited = False
                for r, way in enumerate(ways):
                    lhsT = stat_sb[:, r * 128 : (r + 1) * 128]

                    if way == "P":
                        # PSUM-direct: both reductions read PSUM, no export.
                        rp = rp_pool.tile([128, 4], f32)
                        for g in range(4):
                            pt = psum_pool.tile([128, GW], f32, tag="pt")
                            for s in range(GW // MMW):
                                c0 = g * GW + s * MMW
                                nc.tensor.matmul(
                                    pt[:, s * MMW : (s + 1) * MMW],
                                    lhsT,
                                    mov_sb[:, c0 : c0 + MMW],
                                    start=True,
                                    stop=True,
                                )
                            csl = colacc[:, g * GW : (g + 1) * GW]
                            nc.vector.tensor_tensor(csl, pt[:], csl, MIN)
                            scr = scr_pool.tile([128, N // 2], f16)
                            nc.vector.tensor_tensor_reduce(
                                scr[:, : GW // 2],
                                pt[:, : GW // 2],
                                pt[:, GW // 2 :],
                                1.0,
                                BIG,
                                MIN,
                                MIN,
                                rp[:, g : g + 1],
                            )
                        nc.vector.tensor_reduce(
                            rowmins[:, r : r + 1], rp[:], X, MIN
                        )
                        continue

                    # Exported tiles.  First exported tile of each
                    # accumulator writes it directly (free init).
                    init_b = False
                    if way == "E" and r == 0:
                        dst = colacc
                    elif way in "GFV" and not b_inited:
                        dst = colaccB
                        b_inited = True
                        init_b = True
                    else:
                        dst = ckt_pool.tile([128, N], f16, tag="ck16")

                    for g in range(4):
                        pt = psum_pool.tile([128, GW], f32, tag="pt")
                        for s in range(GW // MMW):
                            c0 = g * GW + s * MMW
                            nc.tensor.matmul(
                                pt[:, s * MMW : (s + 1) * MMW],
                                lhsT,
                                mov_sb[:, c0 : c0 + MMW],
                                start=True,
                                stop=True,
                            )
                        dsl = dst[:, g * GW : (g + 1) * GW]
                        if way == "V":
                            nc.vector.tensor_copy(dsl, pt[:])
                        else:
                            nc.scalar.copy(dsl, pt[:])

                    # colmin merge
                    if way == "E":
                        if r > 0:
                            nc.vector.tensor_tensor(colacc[:], dst[:], colacc[:], MIN)
                    elif not init_b:  # G/F/V
                        nc.gpsimd.tensor_tensor(colaccB[:], dst[:], colaccB[:], MIN)

                    # rowmin
                    if way == "F":
                        nc.gpsimd.tensor_reduce(
                            rowmins[:, r : r + 1], dst[:], X, MIN
                        )
                    elif rowmode == "ttr":
                        scr = scr_pool.tile([128, N // 2], f16)
                        nc.vector.tensor_tensor_reduce(
                            scr[:],
                            dst[:, : N // 2],
                            dst[:, N // 2 :],
                            1.0,
                            BIG,
                            MIN,
                            MIN,
                            rowmins[:, r : r + 1],
                        )
                    elif rowmode == "ttr_bc":
                        # qr.py-style: dummy broadcast out, real accum
                        scr = scr_pool.tile([128, 1], f16, tag="scrbc")
                        nc.vector.tensor_tensor_reduce(
                            scr[:].broadcast_to((128, N // 2)),
                            dst[:, : N // 2],
                            dst[:, N // 2 :],
                            1.0,
                            BIG,
                            MIN,
                            MIN,
                            rowmins[:, r : r + 1],
                        )
                    elif rowmode == "ttr_add":
                        scr = scr_pool.tile([128, 1], f16, tag="scrbc")
                        nc.vector.tensor_tensor_reduce(
                            scr[:].broadcast_to((128, N // 2)),
                            dst[:, : N // 2],
                            dst[:, N // 2 :],
                            1.0,
                            0.0,
                            MIN,
                            mybir.AluOpType.add,
                            rowmins[:, r : r + 1],
                        )
                    elif rowmode == "ttr_rp":
                        scr = scr_pool.tile([128, N // 2], f16)
                        rp = rp_pool.tile([128, 4], f32)
                        nc.vector.tensor_tensor_reduce(
                            scr[:],
                            dst[:, : N // 2],
                            dst[:, N // 2 :],
                            1.0,
                            BIG,
                            MIN,
                            MIN,
                            rp[:, 0:1],
                        )
                        nc.vector.tensor_reduce(
                            rowmins[:, r : r + 1], rp[:, 0:1], X, MIN
                        )
                    elif rowmode == "pooltest":
                        # timing probe only: row-MAX via pool (wrong values)
                        nc.vector.pool_max(rowmins[:, r : r + 1], dst[:])
                    else:  # tree
                        scr = scr_pool.tile([128, N // 2], f16)
                        nc.vector.tensor_tensor(
                            scr[:], dst[:, : N // 2], dst[:, N // 2 :], MIN
                        )
                        scr2 = scr_pool.tile([128, N // 4], f16, tag="scr2")
                        nc.vector.tensor_tensor(
                            scr2[:], scr[:, : N // 4], scr[:, N // 4 :], MIN
                        )
                        scr3 = scr_pool.tile([128, N // 8], f16, tag="scr3")
                        nc.vector.tensor_tensor(
                            scr3[:], scr2[:, : N // 8], scr2[:, N // 8 :], MIN
                        )
                        scr4 = scr_pool.tile([128, N // 16], f16, tag="scr4")
                        nc.vector.tensor_tensor(
                            scr4[:], scr3[:, : N // 16], scr3[:, N // 16 :], MIN
                        )
                        scr5 = scr_pool.tile([128, N // 32], f16, tag="scr5")
                        nc.vector.tensor_tensor(
                            scr5[:], scr4[:, : N // 32], scr4[:, N // 32 :], MIN
                        )
                        nc.vector.tensor_reduce(
                            rowmins[:, r : r + 1], scr5[:], X, MIN
                        )

                # Fold the fp16 accumulator into the fp32 one; chunked so it
                # pipelines with the finale transposes.  The finale
                # transposes read fp32 (PE transpose out dtype must match).
                # The no-B cast-copy runs on ACT, which has slack.
                for q in range(4):
                    sl = slice(q * GW, (q + 1) * GW)
                    if uses_b:
                        nc.vector.tensor_tensor(
                            colaccB[:, sl], colacc[:, sl], colaccB[:, sl], MIN
                        )
                    else:
                        nc.scalar.copy(colaccB[:, sl], colacc[:, sl])

                # --- colmin finale: cross-partition reduce of colaccB ---
                # PE transpose of 128x128 blocks, packed min-reduce
                # 4 blocks per PSUM tile on DVE.
                for j in range(NT // 4):
                    pf = psum_pool.tile([128, GW], f32, tag="pt")
                    for kk in range(4):
                        t = 4 * j + kk
                        nc.tensor.matmul(
                            pf[:, kk * 128 : (kk + 1) * 128],
                            colaccB[:, t * 128 : (t + 1) * 128],
                            ident_sb[:],
                            is_transpose=True,
                            start=True,
                            stop=True,
                        )
                    nc.vector.tensor_reduce(
                        colmins[:, 4 * j : 4 * j + 4],
                        pf[:, :512].rearrange("p (b f) -> p b f", b=4),
                        X,
                        MIN,
                    )
            nc.sync.dma_start(rowmin_o, rowmins[:])
            nc.sync.dma_start(colmin_o, colmins[:])
    nc.compile()
    return nc


def _get_nc():
    if "nc" not in _CACHE:
        _CACHE["nc"] = _build_nc()
    return _CACHE["nc"]


def _split16(x):
    hi = x.astype(np.float16)
    lo = (x - hi.astype(np.float32)).astype(np.float16)
    return hi, lo


def _prep_batch(p, t):
    """p, t: [3, N] fp32 -> (S [K, N] fp16 stationary, M [K, N] fp16 moving)
    with d2[i, j] = sum_k S[k, i] * M[k, j] to ~1e-6 absolute."""
    p2 = (p * p).sum(axis=0)
    t2 = (t * t).sum(axis=0)
    S = np.empty((K, N), np.float16)
    M = np.empty((K, N), np.float16)
    S[0], S[1] = _split16(p2)
    M[0] = 1.0
    M[1] = 1.0
    S[2] = 1.0
    S[3] = 1.0
    M[2], M[3] = _split16(t2)
    for d in range(D):
        ah, al = _split16(-2.0 * p[d])
        bh, bl = _split16(t[d])
        base = 4 + 4 * d
        S[base + 0] = ah
        M[base + 0] = bh
        S[base + 1] = ah
        M[base + 1] = bl
        S[base + 2] = al
        M[base + 2] = bh
        S[base + 3] = al
        M[base + 3] = bl
    return S, M


def _make_in_maps(pred, target):
    pred = np.asarray(pred, dtype=np.float32)
    target = np.asarray(target, dtype=np.float32)
    ident = np.eye(128, dtype=np.float32)
    in_maps = []
    for c in range(NCORES):
        b, h = divmod(c, 2)
        S, M = _prep_batch(pred[b], target[b])
        in_maps.append(
            {
                "stat": np.ascontiguousarray(S[:, h * HALF : (h + 1) * HALF]),
                "mov": M,
                "ident": ident,
            }
        )
    return in_maps


def _finish(results):
    """results: list of 8 dicts with 'rowmin' [128, RT] f32 and
    'colmin' [128, NT] f32 (colmin[p, t] = min_i d2[i, 128*t + p])."""
    row_total = 0.0
    col_total = 0.0
    for b in range(B):
        colparts = []
        for h in range(2):
            out = results[2 * b + h]
            rm = np.asarray(out["rowmin"], dtype=np.float32)  # [128, RT]
            # row index within half = r*128 + p -> transpose to [RT, 128]
            rd2 = np.maximum(rm.T.reshape(-1), 0.0)
            row_total += np.sqrt(rd2, dtype=np.float64).sum()
            # column j = 128*t + p -> transpose [NT, 128] then flatten
            colparts.append(
                np.asarray(out["colmin"], dtype=np.float32).T.reshape(-1)
            )
        cd2 = np.maximum(np.minimum(colparts[0], colparts[1]), 0.0)
        col_total += np.sqrt(cd2, dtype=np.float64).sum()
    loss = row_total / (B * N) + col_total / (B * N)
    return np.array(loss, dtype=np.float32)


def _run(in_maps, trace=False, nc=None):
    from concourse.bass_utils import run_bass_kernel_spmd

    if nc is None:
        nc = _get_nc()
    res = run_bass_kernel_spmd(
        nc, in_maps, list(range(NCORES)), trace=trace
    )
    return res


def kernel(pred, target):
    res = _run(_make_in_maps(pred, target), trace=False)
    return _finish(res.results)
